# revision 5
# baseline (speedup 1.0000x reference)
"""GQA attention layer (B=2, L=2048, D=4096, H=32, KH=8, HD=128) on 8 TRN2 cores.

v2: bf16 everywhere (weights/activations; fp32 PSUM accumulation),
DMA-friendly host-prepped layouts (big contiguous per-partition lines),
qT kept resident in SBUF (no DRAM roundtrip), per-head AllToAll overlapped
with attention compute, software-pipelined emission so PE never waits on
Act/DVE round trips, causal diagonal trimming.

Sharding: tensor-parallel over KV heads (1 kv head + 4 q heads per core),
AllToAll to redistribute attention outputs token-wise, each core runs the
output projection for its 512-token slice. Host concatenates slices.
"""
import numpy as np
import ml_dtypes

import concourse.bass as bass
import concourse.mybir as mybir
import concourse.tile as tile
from concourse import bacc
from concourse.bass_utils import run_bass_kernel_spmd

F32 = mybir.dt.float32
F32R = mybir.dt.float32r
BF = mybir.dt.bfloat16
AF = mybir.ActivationFunctionType
MUL = mybir.AluOpType.mult
SUB = mybir.AluOpType.subtract
ADD = mybir.AluOpType.add

B, L, D = 2, 2048, 4096
H, KH, HD = 32, 8, 128
T = B * L              # 4096 tokens
NC_ = 8                # cores
QH = H // NC_          # 4 q heads per core
NT = T // 128          # 32 token tiles
QB = 512               # q block
EPS = 1e-5
ROPE_BASE = 1000000.0
NPBF = ml_dtypes.bfloat16

_CACHE = {}


def _build():
    nc = bacc.Bacc("TRN2", target_bir_lowering=False, debug=False, num_devices=NC_)

    xp = nc.dram_tensor("xp", [128, NT, 32, 128], BF, kind="ExternalInput").ap()
    wqkv = nc.dram_tensor("wqkv", [128, 32, 768], BF, kind="ExternalInput").ap()
    ropep = nc.dram_tensor("ropep", [128, NT, 5, 256], BF, kind="ExternalInput").ap()
    patd = nc.dram_tensor("patd", [128, 128], BF, kind="ExternalInput").ap()
    identd = nc.dram_tensor("identd", [128, 128], BF, kind="ExternalInput").ap()
    onesc_d = nc.dram_tensor("onesc", [128, 1], BF, kind="ExternalInput").ap()
    onesr_d = nc.dram_tensor("onesr", [1, 128], BF, kind="ExternalInput").ap()
    wop = nc.dram_tensor("wop", [8, QH, 128, 8, 512], BF, kind="ExternalInput").ap()
    out = nc.dram_tensor("out", [T // NC_, D], F32, kind="ExternalOutput").ap()

    with tile.TileContext(nc) as tc:
        with (
            tc.tile_pool(name="const", bufs=1) as cp,
            tc.tile_pool(name="dram", bufs=1, space="DRAM") as dramp,
            tc.tile_pool(name="kv", bufs=1) as kvp,
            tc.tile_pool(name="at", bufs=1) as atp,
        ):
            ident_sb = cp.tile([128, 128], BF)
            nc.sync.dma_start(ident_sb[:], identd)
            pat_sb = cp.tile([128, 128], BF)
            nc.sync.dma_start(pat_sb[:], patd)
            onesc_sb = cp.tile([128, 1], BF)
            nc.sync.dma_start(onesc_sb[:], onesc_d)
            onesr_sb = cp.tile([1, 128], BF)
            nc.sync.dma_start(onesr_sb[:], onesr_d)

            kT_sb = kvp.tile([128, T], BF)            # [hd, tok]
            v_sb = kvp.tile([128, NT, 128], BF)       # [tok%128, tile, hd]
            qT_sb = kvp.tile([128, QH, T], BF)        # [hd, head, tok]

            a2a_in = [dramp.tile([NC_, 128, QB], BF, name=f"a2ain{h}")
                      for h in range(QH)]
            a2a_out = [dramp.tile([NC_, 128, QB], BF, name=f"a2aout{h}")
                       for h in range(QH)]
            at_sb = [atp.tile([128, NC_, QB], BF, name=f"at{h}") for h in range(QH)]

            # ---------------- phase 1: projections + norm + rope ----------
            with (
                tc.tile_pool(name="wts", bufs=1) as wp,
                tc.tile_pool(name="px", bufs=3) as px,
                tc.tile_pool(name="p1", bufs=2) as p1,
                tc.tile_pool(name="ps1", bufs=2, space="PSUM") as ps1,
                tc.tile_pool(name="pst", bufs=4, space="PSUM") as pst,
            ):
                wqkv_sb = wp.tile([128, 32, 768], BF)
                for jc in range(4):
                    nc.sync.dma_start(wqkv_sb[:, 8 * jc:8 * (jc + 1), :],
                                      wqkv[:, 8 * jc:8 * (jc + 1), :])

                def transpose_flush(qkr_prev, i_prev):
                    with nc.allow_low_precision(reason="pure transpose"):
                        for hh in range(5):
                            pt = pst.tile([128, 128], BF, tag="pt", name="pt")
                            nc.tensor.transpose(pt[:], qkr_prev[:, hh, :], ident_sb[:])
                            if hh < QH:
                                nc.scalar.copy(
                                    qT_sb[:, hh, 128 * i_prev:128 * (i_prev + 1)], pt[:])
                            else:
                                nc.scalar.copy(
                                    kT_sb[:, 128 * i_prev:128 * (i_prev + 1)], pt[:])

                prev = None
                for i in range(NT):
                    xt = px.tile([128, 32, 128], BF, tag="xt", name="xt")
                    nc.sync.dma_start(xt[:], xp[:, i, :, :])
                    rp = p1.tile([128, 5, 256], BF, tag="rp", name="rp")
                    nc.sync.dma_start(rp[:], ropep[:, i, :, :])
                    psq = ps1.tile([128, 512], F32, tag="psq", name="psq")
                    pskv = ps1.tile([128, 256], F32, tag="pskv", name="pskv")
                    for j in range(32):
                        xs = xt[:, j, :]
                        nc.tensor.matmul(psq[:], xs, wqkv_sb[:, j, 0:512],
                                         start=(j == 0), stop=(j == 31))
                        nc.tensor.matmul(pskv[:], xs, wqkv_sb[:, j, 512:768],
                                         start=(j == 0), stop=(j == 31))
                    if prev is not None:
                        transpose_flush(*prev)
                    # Act engine: move q/k/v out of PSUM (bf16)
                    qkc = p1.tile([128, 5, 128], BF, tag="qkc", name="qkc")
                    nc.scalar.copy(qkc[:, 0:4, :], psq[:])
                    nc.scalar.copy(qkc[:, 4, :], pskv[:, 0:128])
                    nc.scalar.copy(v_sb[:, i, :], pskv[:, 128:256])
                    # DVE: per-head RMS stats
                    sq = p1.tile([128, 5, 128], BF, tag="sq", name="sq")
                    nc.vector.tensor_tensor(sq[:], qkc[:], qkc[:], MUL)
                    ssq = p1.tile([128, 5], F32, tag="ssq", name="ssq")
                    nc.vector.reduce_sum(ssq[:], sq[:], axis=mybir.AxisListType.X)
                    var = p1.tile([128, 5], F32, tag="var", name="var")
                    nc.vector.tensor_scalar(var[:], ssq[:], 1.0 / HD, EPS, MUL, ADD)
                    rms = p1.tile([128, 5], F32, tag="rms", name="rms")
                    nc.scalar.activation(rms[:], var[:], AF.Sqrt)
                    inv = p1.tile([128, 5], F32, tag="inv", name="inv")
                    nc.vector.reciprocal(inv[:], rms[:])
                    qkn = p1.tile([128, 5, 128], BF, tag="qkn", name="qkn")
                    for hh in range(5):
                        nc.vector.tensor_scalar_mul(qkn[:, hh, :], qkc[:, hh, :],
                                                    inv[:, hh:hh + 1])
                    # rope over all 5 heads at once; tables carry norm-w
                    # (and softmax scale for q)
                    qkr = p1.tile([128, 5, 128], BF, tag="qkr", name="qkr")
                    ta = p1.tile([128, 5, 64], BF, tag="ta", name="ta")
                    tb = p1.tile([128, 5, 64], BF, tag="tb", name="tb")
                    x1 = qkn[:, :, 0:64]
                    x2 = qkn[:, :, 64:128]
                    nc.vector.tensor_tensor(ta[:], x1, rp[:, :, 0:64], MUL)
                    nc.vector.tensor_tensor(tb[:], x2, rp[:, :, 64:128], MUL)
                    nc.vector.tensor_tensor(qkr[:, :, 0:64], ta[:], tb[:], SUB)
                    nc.vector.tensor_tensor(ta[:], x2, rp[:, :, 128:192], MUL)
                    nc.vector.tensor_tensor(tb[:], x1, rp[:, :, 192:256], MUL)
                    nc.vector.tensor_tensor(qkr[:, :, 64:128], ta[:], tb[:], ADD)
                    prev = (qkr, i)
                transpose_flush(*prev)

            # ---------------- phase 2: attention + per-head AllToAll --------
            with (
                tc.tile_pool(name="p2", bufs=3) as p2,
                tc.tile_pool(name="ps2", bufs=3, space="PSUM") as ps2,
                tc.tile_pool(name="pso", bufs=2, space="PSUM") as pso,
                tc.tile_pool(name="psbp", bufs=1, space="PSUM") as psbp,
            ):
                def emit_sum_out(st):
                    (pT, col0, ktile, kt, nkt, pso_s, pso_o) = st
                    nc.tensor.matmul(pso_s[:, col0:QB], onesc_sb[:], pT[:, col0:QB],
                                     start=(kt == 0), stop=(kt == nkt - 1))
                    nc.tensor.matmul(pso_o[:, col0:QB], v_sb[:, ktile, :],
                                     pT[:, col0:QB],
                                     start=(kt == 0), stop=(kt == nkt - 1))

                def emit_tail(st):
                    (h, j, pso_s, pso_o) = st
                    recf = p2.tile([1, QB], F32, tag="recf", name="recf")
                    nc.vector.reciprocal_approx_fast(recf[:], pso_s[:])
                    recb = p2.tile([1, QB], BF, tag="recb", name="recb")
                    with nc.allow_low_precision(reason="bf16 softmax denominator"):
                        nc.vector.tensor_copy(out=recb[:], in_=recf[:])
                        psb = psbp.tile([128, QB], F32, tag="psb", name="psb")
                        nc.tensor.matmul(psb[:], onesr_sb[:], recb[:],
                                         start=True, stop=True)
                    bcs = p2.tile([128, QB], BF, tag="bcs", name="bcs")
                    nc.scalar.copy(bcs[:], psb[:])
                    attn = p2.tile([128, QB], BF, tag="attn", name="attn")
                    nc.vector.tensor_tensor(attn[:], pso_o[:], bcs[:], MUL)
                    nc.sync.dma_start(a2a_in[h][j, :, :], attn[:])

                for h in range(QH):
                    pend_so = None
                    pend_tail = None
                    for b in range(B):
                        for qb in range(4):
                            nkt = 4 * qb + 4
                            j = 4 * b + qb
                            q0 = (b * 16 + 4 * qb) * 128
                            qt = qT_sb[:, h, q0:q0 + QB]
                            pso_o = pso.tile([128, QB], F32, tag="o", name="pso_o")
                            pso_s = pso.tile([1, QB], F32, tag="s", name="pso_s")
                            for kt in range(nkt):
                                t = kt - 4 * qb
                                col0 = 128 * t if t > 0 else 0
                                ktile = b * 16 + kt
                                pss = ps2.tile([128, QB], F32, tag="pss", name="pss")
                                nc.tensor.matmul(
                                    pss[:, col0:QB],
                                    kT_sb[:, 128 * ktile:128 * (ktile + 1)],
                                    qt[:, col0:QB], start=True, stop=True)
                                if pend_so is not None:
                                    emit_sum_out(pend_so)
                                pT = p2.tile([128, QB], BF, tag="pT", name="pT")
                                nc.scalar.activation(pT[:, col0:QB], pss[:, col0:QB],
                                                     AF.Exp)
                                if t >= 0:
                                    nc.vector.tensor_tensor(
                                        pT[:, col0:col0 + 128],
                                        pT[:, col0:col0 + 128], pat_sb[:], MUL)
                                pend_so = (pT, col0, ktile, kt, nkt, pso_s, pso_o)
                            if pend_tail is not None:
                                emit_tail(pend_tail)
                            pend_tail = (h, j, pso_s, pso_o)
                    emit_sum_out(pend_so)
                    emit_tail(pend_tail)
                    nc.gpsimd.collective_compute(
                        "AllToAll", mybir.AluOpType.bypass,
                        replica_groups=[list(range(NC_))],
                        ins=[a2a_in[h].opt()], outs=[a2a_out[h].opt()])
                    nc.gpsimd.dma_start(at_sb[h][:],
                                        a2a_out[h].rearrange("s p t -> p s t"))

            # ---------------- phase 4: output projection --------------------
            with (
                tc.tile_pool(name="p4w", bufs=3) as p4w,
                tc.tile_pool(name="p4o", bufs=8) as p4o,
                tc.tile_pool(name="ps4", bufs=2, space="PSUM") as ps4,
            ):
                for oc in range(8):
                    po = [ps4.tile([128, 512], F32, tag=f"po{tt}", name=f"po{tt}")
                          for tt in range(4)]
                    for h in range(QH):
                        wt = p4w.tile([128, 8, 512], BF, tag="wt", name="wt")
                        nc.sync.dma_start(wt[:], wop[oc, h, :, :, :])
                        for s in range(8):
                            k = 8 * h + s
                            for tt in range(4):
                                nc.tensor.matmul(
                                    po[tt][:],
                                    at_sb[h][:, s, 128 * tt:128 * (tt + 1)],
                                    wt[:, s, :], start=(k == 0), stop=(k == 31))
                    for tt in range(4):
                        ob = p4o.tile([128, 512], F32, tag="ob", name="ob")
                        nc.scalar.copy(ob[:], po[tt][:])
                        nc.sync.dma_start(
                            out[128 * tt:128 * (tt + 1), 512 * oc:512 * (oc + 1)],
                            ob[:])

    nc.compile()
    return nc


def _prep(inputs):
    x = np.asarray(inputs["x"], np.float32)
    wq = np.asarray(inputs["wq"], np.float32)
    wk = np.asarray(inputs["wk"], np.float32)
    wv = np.asarray(inputs["wv"], np.float32)
    wo = np.asarray(inputs["wo"], np.float32)
    qw = np.asarray(inputs["q_norm_w"], np.float32)
    kw = np.asarray(inputs["k_norm_w"], np.float32)

    xf = np.ascontiguousarray(x.reshape(T, D))
    # xp[p, i, o, t] = xf[i*128+t, o*128+p]
    xp = np.ascontiguousarray(
        xf.reshape(NT, 128, 32, 128).transpose(3, 0, 2, 1).astype(NPBF))

    half = HD // 2
    inv_freq = 1.0 / (ROPE_BASE ** (np.arange(half, dtype=np.float32) / half))
    pos = np.arange(L, dtype=np.float32)
    ang = pos[:, None] * inv_freq[None, :]
    cos = np.cos(ang).astype(np.float32)
    sin = np.sin(ang).astype(np.float32)
    scale = np.float32(HD ** -0.5)

    def rope_tab(w, s):
        c1 = cos * w[None, 0:half] * s
        s1 = sin * w[None, half:HD] * s
        c2 = cos * w[None, half:HD] * s
        s2 = sin * w[None, 0:half] * s
        return np.concatenate([c1, s1, c2, s2], axis=1)      # [L, 256]

    qtab = rope_tab(qw, scale)
    ktab = rope_tab(kw, np.float32(1.0))
    # rope_all[token, head5, 256]; token = global flat token, pos = token % L
    rope_all = np.empty((T, 5, 256), np.float32)
    posmap = (np.arange(T) % L)
    rope_all[:, 0:4, :] = qtab[posmap][:, None, :]
    rope_all[:, 4, :] = ktab[posmap]
    # ropep[p, i, h5, 256] = rope_all[i*128+p, h5, :]
    ropep = np.ascontiguousarray(
        rope_all.reshape(NT, 128, 5, 256).transpose(1, 0, 2, 3).astype(NPBF))

    kk = np.arange(128)[:, None]
    jj = np.arange(128)[None, :]
    patd = (kk <= jj).astype(NPBF)
    identb = np.eye(128, dtype=NPBF)
    onesc = np.ones((128, 1), NPBF)
    onesr = np.ones((1, 128), NPBF)

    # wop[oc, h, p, s, n] = wo[(4s+h)*128 + p, 512*oc + n]
    wop = np.ascontiguousarray(
        wo.reshape(8, QH, 128, 8, 512).transpose(3, 1, 2, 0, 4).astype(NPBF))

    in_maps = []
    for c in range(NC_):
        wq_c = wq[:, 512 * c:512 * (c + 1)]
        wk_c = wk[:, HD * c:HD * (c + 1)]
        wv_c = wv[:, HD * c:HD * (c + 1)]
        wcat = np.concatenate([wq_c, wk_c, wv_c], axis=1)    # [4096, 768]
        wqkvp = np.ascontiguousarray(
            wcat.reshape(32, 128, 768).transpose(1, 0, 2).astype(NPBF))
        in_maps.append({
            "xp": xp,
            "wqkv": wqkvp,
            "ropep": ropep,
            "patd": patd,
            "identd": identb,
            "onesc": onesc,
            "onesr": onesr,
            "wop": wop,
        })
    return in_maps


def kernel(**inputs) -> np.ndarray:
    if "nc" not in _CACHE:
        _CACHE["nc"] = _build()
    nc = _CACHE["nc"]
    in_maps = _prep(inputs)
    res = run_bass_kernel_spmd(nc, in_maps, list(range(NC_)))
    chunks = [res.results[c]["out"] for c in range(NC_)]
    return np.concatenate(chunks, axis=0).reshape(B, L, D)


# revision 6
# speedup vs baseline: 2.8975x; 2.8975x over previous
"""GQA attention layer (B=2, L=2048, D=4096, H=32, KH=8, HD=128) on 8 TRN2 cores.

v2: bf16 everywhere (weights/activations; fp32 PSUM accumulation),
DMA-friendly host-prepped layouts (big contiguous per-partition lines),
qT kept resident in SBUF (no DRAM roundtrip), per-head AllToAll overlapped
with attention compute, software-pipelined emission so PE never waits on
Act/DVE round trips, causal diagonal trimming.

Sharding: tensor-parallel over KV heads (1 kv head + 4 q heads per core),
AllToAll to redistribute attention outputs token-wise, each core runs the
output projection for its 512-token slice. Host concatenates slices.
"""
import numpy as np
import ml_dtypes

import concourse.bass as bass
import concourse.mybir as mybir
import concourse.tile as tile
from concourse import bacc
from concourse.bass_utils import run_bass_kernel_spmd

F32 = mybir.dt.float32
F32R = mybir.dt.float32r
BF = mybir.dt.bfloat16
AF = mybir.ActivationFunctionType
MUL = mybir.AluOpType.mult
SUB = mybir.AluOpType.subtract
ADD = mybir.AluOpType.add

B, L, D = 2, 2048, 4096
H, KH, HD = 32, 8, 128
T = B * L              # 4096 tokens
NC_ = 8                # cores
QH = H // NC_          # 4 q heads per core
NT = T // 128          # 32 token tiles
QB = 512               # q block
EPS = 1e-5
ROPE_BASE = 1000000.0
NPBF = ml_dtypes.bfloat16

_CACHE = {}


def _build():
    nc = bacc.Bacc("TRN2", target_bir_lowering=False, debug=False, num_devices=NC_)

    xp = nc.dram_tensor("xp", [128, NT, 32, 128], BF, kind="ExternalInput").ap()
    wqkv = nc.dram_tensor("wqkv", [128, 32, 768], BF, kind="ExternalInput").ap()
    ropep = nc.dram_tensor("ropep", [128, NT, 5, 256], BF, kind="ExternalInput").ap()
    patd = nc.dram_tensor("patd", [128, 128], BF, kind="ExternalInput").ap()
    identd = nc.dram_tensor("identd", [128, 128], BF, kind="ExternalInput").ap()
    onesc_d = nc.dram_tensor("onesc", [128, 1], BF, kind="ExternalInput").ap()
    onesr_d = nc.dram_tensor("onesr", [1, 128], BF, kind="ExternalInput").ap()
    wop = nc.dram_tensor("wop", [8, QH, 128, 8, 512], BF, kind="ExternalInput").ap()
    out = nc.dram_tensor("out", [T // NC_, D], F32, kind="ExternalOutput").ap()

    with tile.TileContext(nc) as tc:
        with (
            tc.tile_pool(name="const", bufs=1) as cp,
            tc.tile_pool(name="dram", bufs=1, space="DRAM") as dramp,
            tc.tile_pool(name="kv", bufs=1) as kvp,
            tc.tile_pool(name="at", bufs=1) as atp,
        ):
            ident_sb = cp.tile([128, 128], BF)
            nc.sync.dma_start(ident_sb[:], identd)
            pat_sb = cp.tile([128, 128], BF)
            nc.sync.dma_start(pat_sb[:], patd)
            onesc_sb = cp.tile([128, 1], BF)
            nc.sync.dma_start(onesc_sb[:], onesc_d)
            onesr_sb = cp.tile([1, 128], BF)
            nc.sync.dma_start(onesr_sb[:], onesr_d)

            kT_sb = kvp.tile([128, T], BF)            # [hd, tok]
            v_sb = kvp.tile([128, NT, 128], BF)       # [tok%128, tile, hd]
            qT_sb = kvp.tile([128, QH, T], BF)        # [hd, head, tok]

            a2a_in = [dramp.tile([NC_, 128, QB], BF, name=f"a2ain{h}")
                      for h in range(QH)]
            a2a_out = [dramp.tile([NC_, 128, QB], BF, name=f"a2aout{h}")
                       for h in range(QH)]
            at_sb = [atp.tile([128, NC_, QB], BF, name=f"at{h}") for h in range(QH)]

            # ---------------- phase 1: projections + norm + rope ----------
            with (
                tc.tile_pool(name="wts", bufs=1) as wp,
                tc.tile_pool(name="px", bufs=3) as px,
                tc.tile_pool(name="p1", bufs=2) as p1,
                tc.tile_pool(name="ps1", bufs=2, space="PSUM") as ps1,
                tc.tile_pool(name="pst", bufs=4, space="PSUM") as pst,
            ):
                wqkv_sb = wp.tile([128, 32, 768], BF)
                for jc in range(4):
                    nc.sync.dma_start(wqkv_sb[:, 8 * jc:8 * (jc + 1), :],
                                      wqkv[:, 8 * jc:8 * (jc + 1), :])

                def transpose_flush(qkr_prev, i_prev):
                    with nc.allow_low_precision(reason="pure transpose"):
                        for hh in range(5):
                            pt = pst.tile([128, 128], BF, tag="pt", name="pt")
                            nc.tensor.transpose(pt[:], qkr_prev[:, hh, :], ident_sb[:])
                            if hh < QH:
                                nc.scalar.copy(
                                    qT_sb[:, hh, 128 * i_prev:128 * (i_prev + 1)], pt[:])
                            else:
                                nc.scalar.copy(
                                    kT_sb[:, 128 * i_prev:128 * (i_prev + 1)], pt[:])

                prev = None
                for i in range(NT):
                    xt = px.tile([128, 32, 128], BF, tag="xt", name="xt")
                    nc.sync.dma_start(xt[:], xp[:, i, :, :])
                    rp = p1.tile([128, 5, 256], BF, tag="rp", name="rp")
                    nc.sync.dma_start(rp[:], ropep[:, i, :, :])
                    psq = ps1.tile([128, 512], F32, tag="psq", name="psq")
                    pskv = ps1.tile([128, 256], F32, tag="pskv", name="pskv")
                    for j in range(32):
                        xs = xt[:, j, :]
                        nc.tensor.matmul(psq[:], xs, wqkv_sb[:, j, 0:512],
                                         start=(j == 0), stop=(j == 31))
                        nc.tensor.matmul(pskv[:], xs, wqkv_sb[:, j, 512:768],
                                         start=(j == 0), stop=(j == 31))
                    if prev is not None:
                        transpose_flush(*prev)
                    # Act engine: move q/k/v out of PSUM (bf16)
                    qkc = p1.tile([128, 5, 128], BF, tag="qkc", name="qkc")
                    nc.scalar.copy(qkc[:, 0:4, :], psq[:])
                    nc.scalar.copy(qkc[:, 4, :], pskv[:, 0:128])
                    nc.scalar.copy(v_sb[:, i, :], pskv[:, 128:256])
                    # DVE: per-head RMS stats
                    sq = p1.tile([128, 5, 128], BF, tag="sq", name="sq")
                    nc.vector.tensor_tensor(sq[:], qkc[:], qkc[:], MUL)
                    ssq = p1.tile([128, 5], F32, tag="ssq", name="ssq")
                    nc.vector.reduce_sum(ssq[:], sq[:], axis=mybir.AxisListType.X)
                    var = p1.tile([128, 5], F32, tag="var", name="var")
                    nc.vector.tensor_scalar(var[:], ssq[:], 1.0 / HD, EPS, MUL, ADD)
                    rms = p1.tile([128, 5], F32, tag="rms", name="rms")
                    nc.scalar.activation(rms[:], var[:], AF.Sqrt)
                    inv = p1.tile([128, 5], F32, tag="inv", name="inv")
                    nc.vector.reciprocal(inv[:], rms[:])
                    qkn = p1.tile([128, 5, 128], BF, tag="qkn", name="qkn")
                    for hh in range(5):
                        nc.vector.tensor_scalar_mul(qkn[:, hh, :], qkc[:, hh, :],
                                                    inv[:, hh:hh + 1])
                    # rope over all 5 heads at once; tables carry norm-w
                    # (and softmax scale for q)
                    qkr = p1.tile([128, 5, 128], BF, tag="qkr", name="qkr")
                    ta = p1.tile([128, 5, 64], BF, tag="ta", name="ta")
                    tb = p1.tile([128, 5, 64], BF, tag="tb", name="tb")
                    x1 = qkn[:, :, 0:64]
                    x2 = qkn[:, :, 64:128]
                    nc.vector.tensor_tensor(ta[:], x1, rp[:, :, 0:64], MUL)
                    nc.vector.tensor_tensor(tb[:], x2, rp[:, :, 64:128], MUL)
                    nc.vector.tensor_tensor(qkr[:, :, 0:64], ta[:], tb[:], SUB)
                    nc.vector.tensor_tensor(ta[:], x2, rp[:, :, 128:192], MUL)
                    nc.vector.tensor_tensor(tb[:], x1, rp[:, :, 192:256], MUL)
                    nc.vector.tensor_tensor(qkr[:, :, 64:128], ta[:], tb[:], ADD)
                    prev = (qkr, i)
                transpose_flush(*prev)

            # ---------------- phase 2: attention + per-head AllToAll --------
            with (
                tc.tile_pool(name="p2", bufs=3) as p2,
                tc.tile_pool(name="ps2", bufs=3, space="PSUM") as ps2,
                tc.tile_pool(name="pso", bufs=2, space="PSUM") as pso,
                tc.tile_pool(name="psbp", bufs=1, space="PSUM") as psbp,
            ):
                def emit_sum_out(st):
                    (pT, col0, ktile, kt, nkt, pso_s, pso_o) = st
                    nc.tensor.matmul(pso_s[:, col0:QB], onesc_sb[:], pT[:, col0:QB],
                                     start=(kt == 0), stop=(kt == nkt - 1))
                    nc.tensor.matmul(pso_o[:, col0:QB], v_sb[:, ktile, :],
                                     pT[:, col0:QB],
                                     start=(kt == 0), stop=(kt == nkt - 1))

                def emit_tail(st):
                    (h, j, pso_s, pso_o) = st
                    recf = p2.tile([1, QB], F32, tag="recf", name="recf")
                    nc.vector.reciprocal_approx_fast(recf[:], pso_s[:])
                    recb = p2.tile([1, QB], BF, tag="recb", name="recb")
                    with nc.allow_low_precision(reason="bf16 softmax denominator"):
                        nc.vector.tensor_copy(out=recb[:], in_=recf[:])
                        psb = psbp.tile([128, QB], F32, tag="psb", name="psb")
                        nc.tensor.matmul(psb[:], onesr_sb[:], recb[:],
                                         start=True, stop=True)
                    bcs = p2.tile([128, QB], BF, tag="bcs", name="bcs")
                    nc.scalar.copy(bcs[:], psb[:])
                    attn = p2.tile([128, QB], BF, tag="attn", name="attn")
                    nc.vector.tensor_tensor(attn[:], pso_o[:], bcs[:], MUL)
                    nc.sync.dma_start(a2a_in[h][j, :, :], attn[:])

                for h in range(QH):
                    pend_so = None
                    pend_tail = None
                    for b in range(B):
                        for qb in range(4):
                            nkt = 4 * qb + 4
                            j = 4 * b + qb
                            q0 = (b * 16 + 4 * qb) * 128
                            qt = qT_sb[:, h, q0:q0 + QB]
                            pso_o = pso.tile([128, QB], F32, tag="o", name="pso_o")
                            pso_s = pso.tile([1, QB], F32, tag="s", name="pso_s")
                            for kt in range(nkt):
                                t = kt - 4 * qb
                                col0 = 128 * t if t > 0 else 0
                                ktile = b * 16 + kt
                                pss = ps2.tile([128, QB], F32, tag="pss", name="pss")
                                nc.tensor.matmul(
                                    pss[:, col0:QB],
                                    kT_sb[:, 128 * ktile:128 * (ktile + 1)],
                                    qt[:, col0:QB], start=True, stop=True)
                                if pend_so is not None:
                                    emit_sum_out(pend_so)
                                pT = p2.tile([128, QB], BF, tag="pT", name="pT")
                                nc.scalar.activation(pT[:, col0:QB], pss[:, col0:QB],
                                                     AF.Exp)
                                if t >= 0:
                                    nc.vector.tensor_tensor(
                                        pT[:, col0:col0 + 128],
                                        pT[:, col0:col0 + 128], pat_sb[:], MUL)
                                pend_so = (pT, col0, ktile, kt, nkt, pso_s, pso_o)
                            if pend_tail is not None:
                                emit_tail(pend_tail)
                            pend_tail = (h, j, pso_s, pso_o)
                    emit_sum_out(pend_so)
                    emit_tail(pend_tail)
                    nc.gpsimd.collective_compute(
                        "AllToAll", mybir.AluOpType.bypass,
                        replica_groups=[list(range(NC_))],
                        ins=[a2a_in[h].opt()], outs=[a2a_out[h].opt()])
                    eng = nc.scalar if h == QH - 1 else nc.sync
                    eng.dma_start(at_sb[h][:],
                                  a2a_out[h].rearrange("s p t -> p s t"))

            # ---------------- phase 4: output projection --------------------
            with (
                tc.tile_pool(name="p4w", bufs=3) as p4w,
                tc.tile_pool(name="p4o", bufs=8) as p4o,
                tc.tile_pool(name="ps4", bufs=2, space="PSUM") as ps4,
            ):
                for oc in range(8):
                    po = [ps4.tile([128, 512], F32, tag=f"po{tt}", name=f"po{tt}")
                          for tt in range(4)]
                    for h in range(QH):
                        wt = p4w.tile([128, 8, 512], BF, tag="wt", name="wt")
                        nc.sync.dma_start(wt[:], wop[oc, h, :, :, :])
                        for s in range(8):
                            k = 8 * h + s
                            for tt in range(4):
                                nc.tensor.matmul(
                                    po[tt][:],
                                    at_sb[h][:, s, 128 * tt:128 * (tt + 1)],
                                    wt[:, s, :], start=(k == 0), stop=(k == 31))
                    for tt in range(4):
                        ob = p4o.tile([128, 512], F32, tag="ob", name="ob")
                        nc.scalar.copy(ob[:], po[tt][:])
                        nc.sync.dma_start(
                            out[128 * tt:128 * (tt + 1), 512 * oc:512 * (oc + 1)],
                            ob[:])

    nc.compile()
    return nc


def _prep(inputs):
    x = np.asarray(inputs["x"], np.float32)
    wq = np.asarray(inputs["wq"], np.float32)
    wk = np.asarray(inputs["wk"], np.float32)
    wv = np.asarray(inputs["wv"], np.float32)
    wo = np.asarray(inputs["wo"], np.float32)
    qw = np.asarray(inputs["q_norm_w"], np.float32)
    kw = np.asarray(inputs["k_norm_w"], np.float32)

    xf = np.ascontiguousarray(x.reshape(T, D))
    # xp[p, i, o, t] = xf[i*128+t, o*128+p]
    xp = np.ascontiguousarray(
        xf.reshape(NT, 128, 32, 128).transpose(3, 0, 2, 1).astype(NPBF))

    half = HD // 2
    inv_freq = 1.0 / (ROPE_BASE ** (np.arange(half, dtype=np.float32) / half))
    pos = np.arange(L, dtype=np.float32)
    ang = pos[:, None] * inv_freq[None, :]
    cos = np.cos(ang).astype(np.float32)
    sin = np.sin(ang).astype(np.float32)
    scale = np.float32(HD ** -0.5)

    def rope_tab(w, s):
        c1 = cos * w[None, 0:half] * s
        s1 = sin * w[None, half:HD] * s
        c2 = cos * w[None, half:HD] * s
        s2 = sin * w[None, 0:half] * s
        return np.concatenate([c1, s1, c2, s2], axis=1)      # [L, 256]

    qtab = rope_tab(qw, scale)
    ktab = rope_tab(kw, np.float32(1.0))
    # rope_all[token, head5, 256]; token = global flat token, pos = token % L
    rope_all = np.empty((T, 5, 256), np.float32)
    posmap = (np.arange(T) % L)
    rope_all[:, 0:4, :] = qtab[posmap][:, None, :]
    rope_all[:, 4, :] = ktab[posmap]
    # ropep[p, i, h5, 256] = rope_all[i*128+p, h5, :]
    ropep = np.ascontiguousarray(
        rope_all.reshape(NT, 128, 5, 256).transpose(1, 0, 2, 3).astype(NPBF))

    kk = np.arange(128)[:, None]
    jj = np.arange(128)[None, :]
    patd = (kk <= jj).astype(NPBF)
    identb = np.eye(128, dtype=NPBF)
    onesc = np.ones((128, 1), NPBF)
    onesr = np.ones((1, 128), NPBF)

    # wop[oc, h, p, s, n] = wo[(4s+h)*128 + p, 512*oc + n]
    wop = np.ascontiguousarray(
        wo.reshape(8, QH, 128, 8, 512).transpose(3, 1, 2, 0, 4).astype(NPBF))

    in_maps = []
    for c in range(NC_):
        wq_c = wq[:, 512 * c:512 * (c + 1)]
        wk_c = wk[:, HD * c:HD * (c + 1)]
        wv_c = wv[:, HD * c:HD * (c + 1)]
        wcat = np.concatenate([wq_c, wk_c, wv_c], axis=1)    # [4096, 768]
        wqkvp = np.ascontiguousarray(
            wcat.reshape(32, 128, 768).transpose(1, 0, 2).astype(NPBF))
        in_maps.append({
            "xp": xp,
            "wqkv": wqkvp,
            "ropep": ropep,
            "patd": patd,
            "identd": identb,
            "onesc": onesc,
            "onesr": onesr,
            "wop": wop,
        })
    return in_maps


def kernel(**inputs) -> np.ndarray:
    if "nc" not in _CACHE:
        _CACHE["nc"] = _build()
    nc = _CACHE["nc"]
    in_maps = _prep(inputs)
    res = run_bass_kernel_spmd(nc, in_maps, list(range(NC_)))
    chunks = [res.results[c]["out"] for c in range(NC_)]
    return np.concatenate(chunks, axis=0).reshape(B, L, D)


# revision 12
# speedup vs baseline: 3.1533x; 1.0883x over previous
"""GQA attention layer (B=2, L=2048, D=4096, H=32, KH=8, HD=128) on 8 TRN2 cores.

v2: bf16 everywhere (weights/activations; fp32 PSUM accumulation),
DMA-friendly host-prepped layouts (big contiguous per-partition lines),
qT kept resident in SBUF (no DRAM roundtrip), per-head AllToAll overlapped
with attention compute, software-pipelined emission so PE never waits on
Act/DVE round trips, causal diagonal trimming.

Sharding: tensor-parallel over KV heads (1 kv head + 4 q heads per core),
AllToAll to redistribute attention outputs token-wise, each core runs the
output projection for its 512-token slice. Host concatenates slices.
"""
import numpy as np
import ml_dtypes

import concourse.bass as bass
import concourse.mybir as mybir
import concourse.tile as tile
from concourse import bacc
from concourse.bass_utils import run_bass_kernel_spmd

F32 = mybir.dt.float32
F32R = mybir.dt.float32r
BF = mybir.dt.bfloat16
AF = mybir.ActivationFunctionType
MUL = mybir.AluOpType.mult
SUB = mybir.AluOpType.subtract
ADD = mybir.AluOpType.add

B, L, D = 2, 2048, 4096
H, KH, HD = 32, 8, 128
T = B * L              # 4096 tokens
NC_ = 8                # cores
QH = H // NC_          # 4 q heads per core
NT = T // 128          # 32 token tiles
QB = 512               # q block
EPS = 1e-5
ROPE_BASE = 1000000.0
NPBF = ml_dtypes.bfloat16

_CACHE = {}


def _build():
    nc = bacc.Bacc("TRN2", target_bir_lowering=False, debug=False, num_devices=NC_)

    xp = nc.dram_tensor("xp", [128, NT, 32, 128], BF, kind="ExternalInput").ap()
    wqkv = nc.dram_tensor("wqkv", [128, 32, 768], BF, kind="ExternalInput").ap()
    ropep = nc.dram_tensor("ropep", [128, NT, 5, 256], BF, kind="ExternalInput").ap()
    patd = nc.dram_tensor("patd", [128, 128], BF, kind="ExternalInput").ap()
    identd = nc.dram_tensor("identd", [128, 128], BF, kind="ExternalInput").ap()
    onesc_d = nc.dram_tensor("onesc", [128, 1], BF, kind="ExternalInput").ap()
    onesr_d = nc.dram_tensor("onesr", [1, 128], BF, kind="ExternalInput").ap()
    wop = nc.dram_tensor("wop", [8, QH, 128, 8, 512], BF, kind="ExternalInput").ap()
    out = nc.dram_tensor("out", [T // NC_, D], F32, kind="ExternalOutput").ap()

    with tile.TileContext(nc) as tc:
        with (
            tc.tile_pool(name="const", bufs=1) as cp,
            tc.tile_pool(name="dram", bufs=1, space="DRAM") as dramp,
            tc.tile_pool(name="kv", bufs=1) as kvp,
            tc.tile_pool(name="at", bufs=1) as atp,
        ):
            ident_sb = cp.tile([128, 128], BF)
            nc.sync.dma_start(ident_sb[:], identd)
            pat_sb = cp.tile([128, 128], BF)
            nc.sync.dma_start(pat_sb[:], patd)
            onesc_sb = cp.tile([128, 1], BF)
            nc.sync.dma_start(onesc_sb[:], onesc_d)
            onesr_sb = cp.tile([1, 128], BF)
            nc.sync.dma_start(onesr_sb[:], onesr_d)

            kT_sb = kvp.tile([128, T], BF)            # [hd, tok]
            v_sb = kvp.tile([128, NT, 128], BF)       # [tok%128, tile, hd]
            qT_sb = kvp.tile([128, QH, T], BF)        # [hd, head, tok]

            a2a_in = [dramp.tile([NC_, 128, QB], BF, name=f"a2ain{h}")
                      for h in range(QH)]
            a2a_out = [dramp.tile([NC_, 128, QB], BF, name=f"a2aout{h}")
                       for h in range(QH)]
            at_sb = [atp.tile([128, NC_, QB], BF, name=f"at{h}") for h in range(QH)]

            # ---------------- phase 1: projections + norm + rope ----------
            with (
                tc.tile_pool(name="wts", bufs=1) as wp,
                tc.tile_pool(name="px", bufs=3) as px,
                tc.tile_pool(name="p1", bufs=2) as p1,
                tc.tile_pool(name="ps1", bufs=2, space="PSUM") as ps1,
                tc.tile_pool(name="pst", bufs=4, space="PSUM") as pst,
            ):
                wqkv_sb = []
                for jc in range(4):
                    wc = wp.tile([128, 8, 768], BF, name=f"wqkv{jc}")
                    nc.sync.dma_start(wc[:], wqkv[:, 8 * jc:8 * (jc + 1), :])
                    wqkv_sb.append(wc)

                def transpose_flush(qkr_prev, i_prev):
                    with nc.allow_low_precision(reason="pure transpose"):
                        for hh in range(5):
                            pt = pst.tile([128, 128], BF, tag="pt", name="pt")
                            nc.tensor.transpose(pt[:], qkr_prev[:, hh, :], ident_sb[:])
                            if hh < QH:
                                nc.scalar.copy(
                                    qT_sb[:, hh, 128 * i_prev:128 * (i_prev + 1)], pt[:])
                            else:
                                nc.scalar.copy(
                                    kT_sb[:, 128 * i_prev:128 * (i_prev + 1)], pt[:])

                prev = None
                for i in range(NT):
                    xt = px.tile([128, 32, 128], BF, tag="xt", name="xt")
                    nc.sync.dma_start(xt[:], xp[:, i, :, :])
                    rp = p1.tile([128, 5, 256], BF, tag="rp", name="rp")
                    nc.sync.dma_start(rp[:], ropep[:, i, :, :])
                    psq = ps1.tile([128, 512], F32, tag="psq", name="psq")
                    pskv = ps1.tile([128, 256], F32, tag="pskv", name="pskv")
                    for j in range(32):
                        xs = xt[:, j, :]
                        wj = wqkv_sb[j // 8][:, j % 8, :]
                        nc.tensor.matmul(psq[:], xs, wj[:, 0:512],
                                         start=(j == 0), stop=(j == 31))
                        nc.tensor.matmul(pskv[:], xs, wj[:, 512:768],
                                         start=(j == 0), stop=(j == 31))
                    if prev is not None:
                        transpose_flush(*prev)
                    # Act engine: move q/k/v out of PSUM (bf16)
                    qkc = p1.tile([128, 5, 128], BF, tag="qkc", name="qkc")
                    nc.scalar.copy(qkc[:, 0:4, :], psq[:])
                    nc.scalar.copy(qkc[:, 4, :], pskv[:, 0:128])
                    nc.scalar.copy(v_sb[:, i, :], pskv[:, 128:256])
                    # DVE: per-head RMS stats
                    sq = p1.tile([128, 5, 128], BF, tag="sq", name="sq")
                    nc.vector.tensor_tensor(sq[:], qkc[:], qkc[:], MUL)
                    ssq = p1.tile([128, 5], F32, tag="ssq", name="ssq")
                    nc.vector.reduce_sum(ssq[:], sq[:], axis=mybir.AxisListType.X)
                    var = p1.tile([128, 5], F32, tag="var", name="var")
                    nc.vector.tensor_scalar(var[:], ssq[:], 1.0 / HD, EPS, MUL, ADD)
                    rms = p1.tile([128, 5], F32, tag="rms", name="rms")
                    nc.scalar.activation(rms[:], var[:], AF.Sqrt)
                    inv = p1.tile([128, 5], F32, tag="inv", name="inv")
                    nc.vector.reciprocal(inv[:], rms[:])
                    qkn = p1.tile([128, 5, 128], BF, tag="qkn", name="qkn")
                    for hh in range(5):
                        nc.vector.tensor_scalar_mul(qkn[:, hh, :], qkc[:, hh, :],
                                                    inv[:, hh:hh + 1])
                    # rope over all 5 heads at once; tables carry norm-w
                    # (and softmax scale for q)
                    qkr = p1.tile([128, 5, 128], BF, tag="qkr", name="qkr")
                    ta = p1.tile([128, 5, 64], BF, tag="ta", name="ta")
                    tb = p1.tile([128, 5, 64], BF, tag="tb", name="tb")
                    x1 = qkn[:, :, 0:64]
                    x2 = qkn[:, :, 64:128]
                    nc.vector.tensor_tensor(ta[:], x1, rp[:, :, 0:64], MUL)
                    nc.vector.tensor_tensor(tb[:], x2, rp[:, :, 64:128], MUL)
                    nc.vector.tensor_tensor(qkr[:, :, 0:64], ta[:], tb[:], SUB)
                    nc.vector.tensor_tensor(ta[:], x2, rp[:, :, 128:192], MUL)
                    nc.vector.tensor_tensor(tb[:], x1, rp[:, :, 192:256], MUL)
                    nc.vector.tensor_tensor(qkr[:, :, 64:128], ta[:], tb[:], ADD)
                    prev = (qkr, i)
                transpose_flush(*prev)

            # ---------------- phase 2: attention + per-head AllToAll --------
            with (
                tc.tile_pool(name="p2", bufs=3) as p2,
                tc.tile_pool(name="ps2", bufs=3, space="PSUM") as ps2,
                tc.tile_pool(name="pso", bufs=2, space="PSUM") as pso,
                tc.tile_pool(name="psbp", bufs=1, space="PSUM") as psbp,
            ):
                def emit_out(st):
                    (pT, col0, ktile, kt, nkt, acc, pso_o) = st
                    nc.tensor.matmul(pso_o[:, col0:QB], v_sb[:, ktile, :],
                                     pT[:, col0:QB],
                                     start=(kt == 0), stop=(kt == nkt - 1))

                def emit_tail(st):
                    (h, j, acc, pso_o) = st
                    pso_s = pso.tile([1, QB], F32, tag="s", name="pso_s")
                    nc.tensor.matmul(pso_s[:], onesc_sb[:], acc[:],
                                     start=True, stop=True)
                    recf = p2.tile([1, QB], F32, tag="recf", name="recf")
                    nc.vector.reciprocal_approx_fast(recf[:], pso_s[:])
                    recb = p2.tile([1, QB], BF, tag="recb", name="recb")
                    with nc.allow_low_precision(reason="bf16 softmax denominator"):
                        nc.vector.tensor_copy(out=recb[:], in_=recf[:])
                        psb = psbp.tile([128, QB], F32, tag="psb", name="psb")
                        nc.tensor.matmul(psb[:], onesr_sb[:], recb[:],
                                         start=True, stop=True)
                    bcs = p2.tile([128, QB], BF, tag="bcs", name="bcs")
                    nc.scalar.copy(bcs[:], psb[:])
                    attn = p2.tile([128, QB], BF, tag="attn", name="attn", bufs=8)
                    nc.vector.tensor_tensor(attn[:], pso_o[:], bcs[:], MUL)
                    nc.sync.dma_start(a2a_in[h][j, :, :], attn[:])

                for h in range(QH):
                    pend_so = None
                    pend_tail = None
                    for b in range(B):
                        for qb in range(4):
                            nkt = 4 * qb + 4
                            j = 4 * b + qb
                            q0 = (b * 16 + 4 * qb) * 128
                            qt = qT_sb[:, h, q0:q0 + QB]
                            pso_o = pso.tile([128, QB], F32, tag="o", name="pso_o")
                            acc = p2.tile([128, QB], BF, tag="acc", name="acc",
                                          bufs=3)
                            for kt in range(nkt):
                                t = kt - 4 * qb
                                col0 = 128 * t if t > 0 else 0
                                ktile = b * 16 + kt
                                pss = ps2.tile([128, QB], F32, tag="pss", name="pss")
                                nc.tensor.matmul(
                                    pss[:, col0:QB],
                                    kT_sb[:, 128 * ktile:128 * (ktile + 1)],
                                    qt[:, col0:QB], start=True, stop=True)
                                if pend_so is not None:
                                    emit_out(pend_so)
                                pT = p2.tile([128, QB], BF, tag="pT", name="pT")
                                nc.scalar.activation(pT[:, col0:QB], pss[:, col0:QB],
                                                     AF.Exp)
                                if t >= 0:
                                    nc.vector.tensor_tensor(
                                        pT[:, col0:col0 + 128],
                                        pT[:, col0:col0 + 128], pat_sb[:], MUL)
                                with nc.allow_low_precision(
                                        reason="bf16 softmax denominator acc"):
                                    if kt == 0:
                                        nc.vector.tensor_copy(out=acc[:], in_=pT[:])
                                    else:
                                        nc.vector.tensor_tensor(
                                            acc[:, col0:QB], acc[:, col0:QB],
                                            pT[:, col0:QB], ADD)
                                pend_so = (pT, col0, ktile, kt, nkt, acc, pso_o)
                            if pend_tail is not None:
                                emit_tail(pend_tail)
                            pend_tail = (h, j, acc, pso_o)
                    emit_out(pend_so)
                    pend_so = None
                    emit_tail(pend_tail)
                    nc.gpsimd.collective_compute(
                        "AllToAll", mybir.AluOpType.bypass,
                        replica_groups=[list(range(NC_))],
                        ins=[a2a_in[h].opt()], outs=[a2a_out[h].opt()])
                    eng = nc.scalar if h == QH - 1 else nc.sync
                    eng.dma_start(at_sb[h][:],
                                  a2a_out[h].rearrange("s p t -> p s t"))

            # ---------------- phase 4: output projection --------------------
            with (
                tc.tile_pool(name="p4w", bufs=3) as p4w,
                tc.tile_pool(name="p4o", bufs=8) as p4o,
                tc.tile_pool(name="ps4", bufs=2, space="PSUM") as ps4,
            ):
                for oc in range(8):
                    po = [ps4.tile([128, 512], F32, tag=f"po{tt}", name=f"po{tt}")
                          for tt in range(4)]
                    for h in range(QH):
                        wt = p4w.tile([128, 8, 512], BF, tag="wt", name="wt")
                        nc.sync.dma_start(wt[:], wop[oc, h, :, :, :])
                        for s in range(8):
                            k = 8 * h + s
                            for tt in range(4):
                                nc.tensor.matmul(
                                    po[tt][:],
                                    at_sb[h][:, s, 128 * tt:128 * (tt + 1)],
                                    wt[:, s, :], start=(k == 0), stop=(k == 31))
                    for tt in range(4):
                        ob = p4o.tile([128, 512], F32, tag="ob", name="ob")
                        nc.scalar.copy(ob[:], po[tt][:])
                        nc.sync.dma_start(
                            out[128 * tt:128 * (tt + 1), 512 * oc:512 * (oc + 1)],
                            ob[:])

    nc.compile()
    return nc


def _prep(inputs):
    x = np.asarray(inputs["x"], np.float32)
    wq = np.asarray(inputs["wq"], np.float32)
    wk = np.asarray(inputs["wk"], np.float32)
    wv = np.asarray(inputs["wv"], np.float32)
    wo = np.asarray(inputs["wo"], np.float32)
    qw = np.asarray(inputs["q_norm_w"], np.float32)
    kw = np.asarray(inputs["k_norm_w"], np.float32)

    xf = np.ascontiguousarray(x.reshape(T, D))
    # xp[p, i, o, t] = xf[i*128+t, o*128+p]
    xp = np.ascontiguousarray(
        xf.reshape(NT, 128, 32, 128).transpose(3, 0, 2, 1).astype(NPBF))

    half = HD // 2
    inv_freq = 1.0 / (ROPE_BASE ** (np.arange(half, dtype=np.float32) / half))
    pos = np.arange(L, dtype=np.float32)
    ang = pos[:, None] * inv_freq[None, :]
    cos = np.cos(ang).astype(np.float32)
    sin = np.sin(ang).astype(np.float32)
    scale = np.float32(HD ** -0.5)

    def rope_tab(w, s):
        c1 = cos * w[None, 0:half] * s
        s1 = sin * w[None, half:HD] * s
        c2 = cos * w[None, half:HD] * s
        s2 = sin * w[None, 0:half] * s
        return np.concatenate([c1, s1, c2, s2], axis=1)      # [L, 256]

    qtab = rope_tab(qw, scale)
    ktab = rope_tab(kw, np.float32(1.0))
    # rope_all[token, head5, 256]; token = global flat token, pos = token % L
    rope_all = np.empty((T, 5, 256), np.float32)
    posmap = (np.arange(T) % L)
    rope_all[:, 0:4, :] = qtab[posmap][:, None, :]
    rope_all[:, 4, :] = ktab[posmap]
    # ropep[p, i, h5, 256] = rope_all[i*128+p, h5, :]
    ropep = np.ascontiguousarray(
        rope_all.reshape(NT, 128, 5, 256).transpose(1, 0, 2, 3).astype(NPBF))

    kk = np.arange(128)[:, None]
    jj = np.arange(128)[None, :]
    patd = (kk <= jj).astype(NPBF)
    identb = np.eye(128, dtype=NPBF)
    onesc = np.ones((128, 1), NPBF)
    onesr = np.ones((1, 128), NPBF)

    # wop[oc, h, p, s, n] = wo[(4s+h)*128 + p, 512*oc + n]
    wop = np.ascontiguousarray(
        wo.reshape(8, QH, 128, 8, 512).transpose(3, 1, 2, 0, 4).astype(NPBF))

    in_maps = []
    for c in range(NC_):
        wq_c = wq[:, 512 * c:512 * (c + 1)]
        wk_c = wk[:, HD * c:HD * (c + 1)]
        wv_c = wv[:, HD * c:HD * (c + 1)]
        wcat = np.concatenate([wq_c, wk_c, wv_c], axis=1)    # [4096, 768]
        wqkvp = np.ascontiguousarray(
            wcat.reshape(32, 128, 768).transpose(1, 0, 2).astype(NPBF))
        in_maps.append({
            "xp": xp,
            "wqkv": wqkvp,
            "ropep": ropep,
            "patd": patd,
            "identd": identb,
            "onesc": onesc,
            "onesr": onesr,
            "wop": wop,
        })
    return in_maps


def kernel(**inputs) -> np.ndarray:
    if "nc" not in _CACHE:
        _CACHE["nc"] = _build()
    nc = _CACHE["nc"]
    in_maps = _prep(inputs)
    res = run_bass_kernel_spmd(nc, in_maps, list(range(NC_)))
    chunks = [res.results[c]["out"] for c in range(NC_)]
    return np.concatenate(chunks, axis=0).reshape(B, L, D)
